# revision 13
# baseline (speedup 1.0000x reference)
"""Blockwise K/V selector (sparse attention) on 8 Trainium2 NeuronCores.

Full computation on device:
  scores = q . compressed_keys / sqrt(D)  -> softmax -> GQA mean-pool over
  heads -> top-16 blocks (rank trick, no sort) -> one fused indirect-DMA
  gather of the selected K+V 64-row blocks per (b, g) pair.

Sharding: the 16 (b, g) pairs are fully independent; each of the 8 cores
processes 2 pairs (pure data parallel, no collectives).

Data movement (memory roofline is the target, ~3.2 MB/core/iter):
  * q and ck are uploaded PRE-TRANSPOSED ([D, heads] / [D, (pair head n)])
    so the score matmuls need no on-device PE transposes or PSUM copies.
    Scoring stays f32: the pooled-prob gap at the rank-16 boundary is as
    small as 1e-6 on this input, so bf16/fp16 scores would flip blocks.
  * K and V are uploaded as ONE bf16 tensor [PAIRS, 2, S, D]; the gather
    output is written bf16 and up-cast to f32 on the host. bf16 is a pure
    0.4%-max quantization of the gathered values (far below the 2e-2
    tolerance) and halves both gather and store HBM traffic.
  * Per pair a single 128-index indirect DMA (16 rows = 4 KiB bf16 per
    index, the max span one dest partition line supports) gathers K and V
    together; one HWDGE store per pair (SP ring / ACT ring).

Rank trick (latency-optimized; the naive compare-mask chain costs ~10 us
in cross-engine handoffs):
  * M[p, c] = A[c] - A[p] is built directly in PSUM by two accumulating
    matmuls over the same rz-scaled e operand; the two contractions run
    the identical MAC sequence, so diagonal and cross ties are EXACT
    zeros (no ulp-mismatch masks needed).
  * rank[c] = #(M[:, c] < 0) + #(upper-tri ties) via one immediate-scalar
    DVE compare per pair (bf16 out) + gpsimd affine_select tie mask +
    two accumulating PE matmuls against a ones column.
  * top-16 selection matrix -> chunk bases in one matmul, as before.
"""
import os
import numpy as np

B = 4
H = 32
G = 4
HPG = H // G          # 8 heads per query group
PAIRS = 2             # (b, g) pairs per core
N = 128               # number of compressed keys / key blocks
D = 128               # head dim
S = 8192              # kv sequence length
BS = 64               # block size
NSEL = 16             # selected blocks
NCORES = 8
# gather granularity: 16 bf16 rows = 4 KiB per index. The indirect-DMA DGE
# maps one index to one dest SBUF partition, so the per-index span must equal
# one partition line of the dest tile (4 KiB) — larger spans corrupt on HW.
CHUNK = 16
RPB = BS // CHUNK     # chunks per block (4)
NCHUNK = 2 * NSEL * RPB  # 128 chunks per pair: 64 K-chunks then 64 V-chunks
SCALE = 1.0 / float(D) ** 0.5
GH = PAIRS * HPG      # 16 heads handled per core

_CACHE = {}
LAST_RESULT = None    # BassKernelResults of the most recent run (for test.py)


def _build_nc():
    import concourse.bass as bass
    import concourse.bacc as bacc
    import concourse.mybir as mybir
    import concourse.tile as tile

    F32 = mybir.dt.float32
    BF16 = mybir.dt.bfloat16

    nc = bacc.Bacc("TRN2", target_bir_lowering=False, debug=False)

    qt_in = nc.dram_tensor("qt_in", [D, GH], F32, kind="ExternalInput")
    ckt_in = nc.dram_tensor("ckt_in", [D, GH * N], F32, kind="ExternalInput")
    kv_in = nc.dram_tensor("kv_in", [PAIRS, 2, S, D], BF16, kind="ExternalInput")
    # bf16 consts: iotabh (128 cols) | pvec | ones
    cb_in = nc.dram_tensor("cb_in", [128, 130], BF16, kind="ExternalInput")
    # f32 consts: cvec per pair
    cf_in = nc.dram_tensor("cf_in", [128, PAIRS], F32, kind="ExternalInput")
    out_kv = nc.dram_tensor("out_kv", [PAIRS, 2, NSEL * BS, D], BF16,
                            kind="ExternalOutput")

    # flat chunk view for the gather: [(p t c) = 2048 chunks, 2048 elems]
    kv_flat = kv_in[:].rearrange("p t (c r) d -> (p t c) (r d)", r=CHUNK)

    # KREPEAT>1 builds the pipeline several times (serialized by the
    # TileContext exit barrier) so device time can be measured as the
    # marginal wall-clock per repeat. KEMPTY=1 emits no-op contexts for
    # calibrating the barrier cost.
    repeat = int(os.environ.get("KREPEAT", "1"))
    empty = bool(int(os.environ.get("KEMPTY", "0")))
    # KSTAGE (timing ablation only): 1=loads, 5=+scores/exp, 2=all compute,
    # 4=loads+const-idx gathers+stores, 0=full
    stage = int(os.environ.get("KSTAGE", "0"))
    for _rep in range(repeat):
        _emit_once(nc, tc_mod=tile, bassmod=bass, mybirmod=mybir, empty=empty,
                   stage=stage,
                   tensors=(qt_in, ckt_in, kv_flat, cb_in, cf_in, out_kv))

    nc.compile()
    return nc


def _emit_once(nc, tc_mod, bassmod, mybirmod, empty, tensors, stage=0):
    bass = bassmod
    mybir = mybirmod
    tile = tc_mod
    (qt_in, ckt_in, kv_flat, cb_in, cf_in, out_kv) = tensors
    from concourse.masks import make_identity
    F32 = mybir.dt.float32
    BF16 = mybir.dt.bfloat16
    I32 = mybir.dt.int32
    Alu = mybir.AluOpType
    Act = mybir.ActivationFunctionType
    Ax = mybir.AxisListType

    with tile.TileContext(nc) as tc:
        if empty:
            with tc.tile_pool(name="noop", bufs=1) as np_:
                t = np_.tile([1, 1], F32)
                nc.vector.memset(t[:], 0.0)
            return
        with tc.tile_pool(name="consts", bufs=1) as cp, \
             tc.tile_pool(name="work", bufs=1) as wp, \
             tc.tile_pool(name="psum", bufs=1, space="PSUM") as pp:

            # ---- loads: ckt halves on SP ring, q + consts on ACT ring ----
            ckt_sb = cp.tile([D, GH * N], F32)
            for p in range(PAIRS):
                nc.sync.dma_start(
                    out=ckt_sb[:, p * HPG * N:(p + 1) * HPG * N],
                    in_=ckt_in[:, p * HPG * N:(p + 1) * HPG * N])
            qt_sb = cp.tile([D, GH], F32)
            nc.scalar.dma_start(out=qt_sb[:], in_=qt_in[:])
            cb = cp.tile([128, 130], BF16)
            nc.scalar.dma_start(out=cb[:], in_=cb_in[:])
            cf = cp.tile([128, PAIRS], F32)
            nc.scalar.dma_start(out=cf[:], in_=cf_in[:])
            iotabh = cb[:, 0:128]
            pvec = cb[:, 128:129]
            onesb = cb[:, 129:130]
            # on-device consts (gpsimd engine, off the critical DMA path)
            ident = cp.tile([128, 128], F32)
            make_identity(nc, ident[:])
            onesf = cp.tile([HPG, N], F32)
            nc.gpsimd.memset(onesf[:], 1.0)
            monesf = cp.tile([HPG, N], F32)
            nc.gpsimd.memset(monesf[:], -1.0)

            if stage == 1:
                return
            if stage == 4:
                # timing probe: gathers+stores with constant indices
                for p in range(PAIRS):
                    idxc = wp.tile([128, 1], I32)
                    nc.gpsimd.iota(idxc[:], pattern=[[0, 1]], base=p * 1024,
                                   channel_multiplier=1)
                    kvsel = wp.tile([128, NCHUNK * CHUNK * D // 128], BF16,
                                    tag=f"kvsel{p}")
                    nc.gpsimd.indirect_dma_start(
                        out=kvsel[:], out_offset=None, in_=kv_flat,
                        in_offset=bass.IndirectOffsetOnAxis(ap=idxc[:, :1],
                                                            axis=0))
                    eng = nc.sync if p == 0 else nc.scalar
                    eng.dma_start(
                        out=out_kv[p].rearrange("t (s j r) d -> (t s j) (r d)",
                                                j=RPB, r=CHUNK),
                        in_=kvsel[:])
                return

            # ---- scoresT[n, g]: one [128,1] matmul per head, both pairs
            # into one PSUM tile ----
            sc_ps = pp.tile([N, GH], F32, tag="sc")
            for g in range(GH):
                nc.tensor.matmul(
                    out=sc_ps[:, g:g + 1],
                    lhsT=ckt_sb[:, g * N:(g + 1) * N],
                    rhs=qt_sb[:, g:g + 1],
                    start=True, stop=True)

            # ---- softmax numerator, no max-subtraction (scores ~ N(0,1)
            # after scaling; order matches jax to ~1e-7 relative, far below
            # the top-k prob gaps). Per pair so pair 0's chain does not
            # wait on pair 1's ck load. ----
            esb = wp.tile([N, GH], F32)
            for p in range(PAIRS):
                nc.scalar.activation(
                    out=esb[:, p * HPG:(p + 1) * HPG],
                    in_=sc_ps[:, p * HPG:(p + 1) * HPG],
                    func=Act.Exp, scale=SCALE)

            if stage == 5:
                return

            # ---- per pair: e^T via PE, then z, 1/z, rz-scaled e (each
            # tile based at partition 0 — PE operand requirement) ----
            escl = []
            for p in range(PAIRS):
                eT_ps = pp.tile([HPG, N], F32, tag=f"et{p}")
                nc.tensor.transpose(out=eT_ps[:],
                                    in_=esb[:, p * HPG:(p + 1) * HPG],
                                    identity=ident[:])
                z = wp.tile([HPG, 1], F32, tag=f"z{p}")
                nc.vector.tensor_reduce(out=z[:, :1], in_=eT_ps[:],
                                        op=Alu.add, axis=Ax.X)
                rz = wp.tile([HPG, 1], F32, tag=f"rz{p}")
                nc.vector.reciprocal(out=rz[:, :1], in_=z[:, :1])
                esc = wp.tile([HPG, N], F32, tag=f"esc{p}")
                nc.vector.tensor_scalar(
                    out=esc[:], in0=eT_ps[:], scalar1=rz[:, :1], scalar2=None,
                    op0=Alu.mult)
                escl.append(esc)

            # ---- M[p, c] = A[c] - A[p] per pair, built in PSUM by two
            # accumulating matmuls over the SAME esc operand: both run the
            # identical MAC sequence, so diagonal/ties are exact zeros ----
            m_ps = []
            for p in range(PAIRS):
                m = pp.tile([128, 128], F32, tag=f"m{p}")
                nc.tensor.matmul(out=m[:], lhsT=onesf[0:HPG, :],
                                 rhs=escl[p][:], start=True, stop=False)
                nc.tensor.matmul(out=m[:], lhsT=escl[p][:],
                                 rhs=monesf[0:HPG, :], start=False, stop=True)
                m_ps.append(m)

            # ---- rank[c] = #(M[:,c] < 0) + #(exact upper-tri ties):
            # immediate-scalar DVE compares (bf16 out), gpsimd tie mask,
            # two accumulating PE matmuls against the ones column ----
            gtb, tmb = [], []
            for p in range(PAIRS):
                g_ = wp.tile([128, 128], BF16, tag=f"gt{p}")
                nc.vector.tensor_scalar(
                    out=g_[:], in0=m_ps[p][:], scalar1=0.0, scalar2=None,
                    op0=Alu.is_lt)
                t_ = wp.tile([128, 128], BF16, tag=f"tb{p}")
                nc.vector.tensor_scalar(
                    out=t_[:], in0=m_ps[p][:], scalar1=0.0, scalar2=None,
                    op0=Alu.is_equal)
                tm = wp.tile([128, 128], BF16, tag=f"tm{p}")
                nc.gpsimd.affine_select(
                    out=tm[:], in_=t_[:], compare_op=Alu.is_gt, fill=0.0,
                    base=0, pattern=[[1, 128]], channel_multiplier=-1)
                gtb.append(g_)
                tmb.append(tm)
            rank_ps = pp.tile([128, PAIRS], F32, tag="rk")
            for p in range(PAIRS):
                nc.tensor.matmul(out=rank_ps[:, p:p + 1], lhsT=gtb[p][:],
                                 rhs=onesb[:], start=True, stop=False)
                nc.tensor.matmul(out=rank_ps[:, p:p + 1], lhsT=tmb[p][:],
                                 rhs=onesb[:], start=False, stop=True)

            # ---- selection matrix -> chunk bases in one matmul per pair:
            # chunk[c] = sum_p [rank[p] == (c%64)//RPB] * (RPB*p) ----
            chunk_ps = pp.tile([128, PAIRS], F32, tag="ch")
            for p in range(PAIRS):
                sel = wp.tile([128, NCHUNK], BF16, tag=f"sel{p}")
                nc.vector.tensor_scalar(
                    out=sel[:], in0=iotabh[:], scalar1=rank_ps[:, p:p + 1],
                    scalar2=None, op0=Alu.is_equal)
                nc.tensor.matmul(out=chunk_ps[:, p:p + 1], lhsT=sel[:],
                                 rhs=pvec[:], start=True, stop=True)
            idxi = wp.tile([128, PAIRS], I32)
            nc.vector.tensor_tensor(
                out=idxi[:], in0=chunk_ps[:], in1=cf[:], op=Alu.add)

            if stage == 2:
                return
            # ---- fused K+V gather (128 chunks x 4 KiB each) and store;
            # p0 store on SP ring, p1 store on ACT ring ----
            for p in range(PAIRS):
                kvsel = wp.tile([128, NCHUNK * CHUNK * D // 128], BF16,
                                tag=f"kvsel{p}")
                nc.gpsimd.indirect_dma_start(
                    out=kvsel[:], out_offset=None, in_=kv_flat,
                    in_offset=bass.IndirectOffsetOnAxis(ap=idxi[:, p:p + 1],
                                                        axis=0))
                if stage == 3:
                    continue
                eng = nc.sync if p == 0 else nc.scalar
                eng.dma_start(
                    out=out_kv[p].rearrange("t (s j r) d -> (t s j) (r d)",
                                            j=RPB, r=CHUNK),
                    in_=kvsel[:])


def _consts():
    import ml_dtypes
    cb = np.zeros((128, 130), dtype=np.float32)
    c = np.arange(NCHUNK, dtype=np.float32)
    cb[:, 0:128] = ((c % (NSEL * RPB)) // RPB)[None, :]
    cb[:, 128] = float(RPB) * np.arange(128, dtype=np.float32)
    cb[:, 129] = 1.0
    # cvec[c, p] = p * (2*S//CHUNK) + (c // 64) * (S//CHUNK) + c % RPB
    ci = np.arange(128, dtype=np.float32)
    cf = (np.arange(PAIRS, dtype=np.float32)[None, :] * (2 * S // CHUNK)
          + (ci[:, None] // (NSEL * RPB)) * (S // CHUNK)
          + (ci[:, None] % RPB)).astype(np.float32)
    return {"cb_in": cb.astype(ml_dtypes.bfloat16),
            "cf_in": np.ascontiguousarray(cf)}


def _in_maps_from_full(query, compressed_keys, keys, values):
    """Shard + pre-transpose the full inputs into per-core in_maps."""
    import ml_dtypes
    consts = _consts()
    in_maps = []
    for core in range(NCORES):
        bs, gs = [], []
        for j in range(PAIRS):
            f = PAIRS * core + j
            bs.append(f // G)
            gs.append(f % G)
        # qt [D, GH]: column p*HPG+h = q[b_p, g_p*HPG+h, -1, :]
        q_s = np.stack([query[b, g * HPG:(g + 1) * HPG, -1, :]
                        for b, g in zip(bs, gs)])          # [P, HPG, D]
        qt = np.ascontiguousarray(q_s.reshape(GH, D).T)     # [D, GH]
        # ckt [D, GH*N]
        ck_s = np.stack([compressed_keys[b, g * HPG:(g + 1) * HPG]
                         for b, g in zip(bs, gs)])          # [P, HPG, N, D]
        ckt = np.ascontiguousarray(
            ck_s.reshape(GH * N, D).T)                      # [D, GH*N]
        # kv bf16 [P, 2, S, D]
        kv = np.stack([np.stack([keys[b, g], values[b, g]])
                       for b, g in zip(bs, gs)])
        kv = kv.astype(ml_dtypes.bfloat16)
        im = {"qt_in": qt, "ckt_in": ckt, "kv_in": np.ascontiguousarray(kv)}
        im.update(consts)
        in_maps.append(im)
    return in_maps


def kernel(query, compressed_keys, keys, values):
    global LAST_RESULT
    from concourse.bass_utils import run_bass_kernel_spmd

    query = np.asarray(query, dtype=np.float32)
    compressed_keys = np.asarray(compressed_keys, dtype=np.float32)
    keys = np.asarray(keys, dtype=np.float32)
    values = np.asarray(values, dtype=np.float32)

    key = (os.environ.get("KREPEAT", "1"), os.environ.get("KEMPTY", "0"),
           os.environ.get("KSTAGE", "0"))
    if key not in _CACHE:
        _CACHE[key] = _build_nc()
    nc = _CACHE[key]

    in_maps = _in_maps_from_full(query, compressed_keys, keys, values)
    res = run_bass_kernel_spmd(nc, in_maps, list(range(NCORES)))
    LAST_RESULT = res

    sel_k = np.empty((B, G, NSEL * BS, D), dtype=np.float32)
    sel_v = np.empty((B, G, NSEL * BS, D), dtype=np.float32)
    for core in range(NCORES):
        for j in range(PAIRS):
            f = PAIRS * core + j
            b, g = f // G, f % G
            okv = np.asarray(res.results[core]["out_kv"][j])
            sel_k[b, g] = okv[0].astype(np.float32)
            sel_v[b, g] = okv[1].astype(np.float32)
    return sel_k, sel_v
